# revision 41
# baseline (speedup 1.0000x reference)
"""Trainium2 distributed attention kernel for nn_Attention_72095321030782.

B=16, S=1024, DIM=1024, H=16, HD=64. Batch data-parallel over 8 cores
(2 batches/core), no collectives. Per core, per batch (fully
SBUF-resident intermediates, bf16 matmul operands):
  P1: x tile -> bf16 -> DMA-xbar transpose -> QKV matmul (bf16 weights)
      -> QK RMSNorm (DVE, free-dim-broadcast rstd) + 2D RoPE (bf16)
      -> DMA-xbar transpose q/k into [feat, tok] layout; v + ones column.
  P2: per (b,h): S^T = k.q via PE, exp on ACT (3-bank groups, bf16 out),
      PV accumulate with ones-column denominator, normalize via
      ones-matmul broadcast, attnT chunks -> DRAM.
  P3: out = attnT.T @ w_out + b_out, streamed from DRAM.
"""

import math
from contextlib import ExitStack

import numpy as np
import ml_dtypes

import concourse.bass as bass
import concourse.tile as tile
from concourse import bacc, mybir
from concourse.bass_utils import run_bass_kernel_spmd

B, S, DIM, H = 16, 1024, 1024, 16
HD = DIM // H            # 64
RD = HD // 2             # 32 rope halves
FT, PT_LEN = 32, 16
THETA = 10000.0
EPS = 1e-6
NCORES = 8
BL = B // NCORES         # 2 batches per core
T = BL * S               # 2048 tokens per core
TPB = S // 128           # 8 token tiles per batch
PB = H // 2              # 8 head-pair blocks
F32 = mybir.dt.float32
BF16 = mybir.dt.bfloat16
AF = mybir.ActivationFunctionType


def _rope_tables():
    freqs = 1.0 / THETA ** (np.arange(0, RD, 2, dtype=np.float32) / RD)
    t = np.arange(FT, dtype=np.float32) / FT * PT_LEN
    fs = np.einsum('n,f->nf', t, freqs).astype(np.float32)
    fs = np.repeat(fs, 2, axis=-1)                       # [FT, 32]
    fh = np.broadcast_to(fs[:, None, :], (FT, FT, RD))
    fw = np.broadcast_to(fs[None, :, :], (FT, FT, RD))
    f = np.concatenate([fh, fw], axis=-1).reshape(S, HD)
    return np.cos(f).astype(np.float32), np.sin(f).astype(np.float32)


def build_graph():
    nc = bacc.Bacc('TRN2', target_bir_lowering=False, debug=False,
                   num_devices=NCORES)
    x_e = nc.declare_dram_parameter('x', [T, DIM], F32, isOutput=False)
    wq_e = nc.declare_dram_parameter('wq_b', [DIM, 3 * DIM], BF16, isOutput=False)
    wo_e = nc.declare_dram_parameter('wo_b', [DIM, DIM], BF16, isOutput=False)
    bq_e = nc.declare_dram_parameter('bq_b', [1, 3 * DIM], BF16, isOutput=False)
    bo_e = nc.declare_dram_parameter('bo_b', [1, DIM], BF16, isOutput=False)
    # rope tables [128, TPB, 8*HD]: 8-head replicated, sign-folded sin,
    # norm weights (and the k renorm factor 8) folded in; separate q/k.
    cosq_e = nc.declare_dram_parameter('cosq_b', [128, TPB, 8 * HD], BF16,
                                       isOutput=False)
    sinq_e = nc.declare_dram_parameter('sinq_b', [128, TPB, 8 * HD], BF16,
                                       isOutput=False)
    cosk_e = nc.declare_dram_parameter('cosk_b', [128, TPB, 8 * HD], BF16,
                                       isOutput=False)
    sink_e = nc.declare_dram_parameter('sink_b', [128, TPB, 8 * HD], BF16,
                                       isOutput=False)
    out_e = nc.declare_dram_parameter('out', [T, DIM], F32, isOutput=True)

    x_ap = x_e.ap()
    out_ap = out_e.ap()

    with nc.allow_low_precision(reason='bf16 matmul pipeline'), \
         tile.TileContext(nc) as tc, ExitStack() as ctx:
        dram = ctx.enter_context(tc.tile_pool(name='dram', bufs=1, space='DRAM'))
        # attnT per batch: [pb][it][128 feat, 128 tok] blocked
        attnT_d = dram.tile([BL, PB, TPB, 128, 128], BF16)

        const = ctx.enter_context(tc.tile_pool(name='const', bufs=1))
        wq_sb = []
        for d in range(8):
            wt = const.tile([128, 3 * DIM], BF16, tag=f'wq{d}')
            nc.sync.dma_start(wt[:], wq_e.ap()[bass.ts(d, 128), :])
            wq_sb.append(wt)
        wo_sb = []
        for d in range(8):
            wt = const.tile([128, DIM], BF16, tag=f'wo{d}')
            nc.sync.dma_start(wt[:], wo_e.ap()[bass.ts(d, 128), :])
            wo_sb.append(wt)
        bq_sb = const.tile([1, 3 * DIM], BF16)
        nc.sync.dma_start(bq_sb[:], bq_e.ap()[:])
        bo_sb = const.tile([1, DIM], BF16)
        nc.sync.dma_start(bo_sb[:], bo_e.ap()[:])
        onesr = const.tile([1, 128], BF16)
        nc.vector.memset(onesr[:], 1.0)
        rope_sb = {}
        for nm, e in (('cq', cosq_e), ('sq', sinq_e), ('ck', cosk_e),
                      ('sk', sink_e)):
            t = const.tile([128, TPB, 8 * HD], BF16, tag=nm)
            nc.sync.dma_start(t[:], e.ap()[:])
            rope_sb[nm] = t
        ones_f = const.tile([1, HD], F32)
        nc.vector.memset(ones_f[:], 1.0)
        ones_b = const.tile([1, HD], BF16)
        nc.vector.tensor_copy(ones_b[:], ones_f[:])
        eps_t = const.tile([128, 1], F32)
        nc.vector.memset(eps_t[:], HD * EPS)

        # per-batch resident q/k transposed + v (rotate via bufs=1 tags)
        res = ctx.enter_context(tc.tile_pool(name='res', bufs=1))

        def run_p3(bb):
            # out projection for batch bb (emitted between P1 and P2 of the
            # next batch: its dense PE work covers the P1 tail drain and
            # keeps the HAM clock warm into P2)
            with tc.tile_pool(name='atp', bufs=3) as atp, \
                 tc.tile_pool(name='yps', bufs=2, space='PSUM') as yps, \
                 tc.tile_pool(name='ysb', bufs=2) as ysb:
                for it in range(TPB):
                    a_t = []
                    for d in range(8):
                        at = atp.tile([128, 128], BF16, tag=f'at{d}')
                        nc.sync.dma_start(at[:], attnT_d[bb, d, it, :, :])
                        a_t.append(at)
                    y = ysb.tile([128, DIM], F32, tag='y')
                    for nb in range(2):
                        ps = yps.tile([128, 512], F32, tag='yp')
                        nc.tensor.matmul(ps[:], onesr[:],
                                         bo_sb[:, bass.ts(nb, 512)],
                                         start=True, stop=False)
                        for d in range(8):
                            nc.tensor.matmul(ps[:], a_t[d][:],
                                             wo_sb[d][:, bass.ts(nb, 512)],
                                             start=False, stop=(d == 7))
                        nc.scalar.activation(y[:, bass.ts(nb, 512)], ps[:],
                                             AF.Copy)
                    nc.gpsimd.dma_start(
                        out_ap[bb * S + it * 128:bb * S + (it + 1) * 128, :],
                        y[:])

        for b in range(BL):
            # layout [feat_pair, pb, tok] keeps P2's QK moving operand
            # contiguous; DMA transposes land in a tmp and gpsimd relayouts
            qT_all = res.tile([128, PB, TPB, 128], BF16, tag='qT')
            kT_all = res.tile([128, PB, TPB, 128], BF16, tag='kT')
            v_all = res.tile([128, TPB, H, HD + 1], BF16, tag='v')

            # ---- P1: QKV + norm + rope for this batch ----
            with tc.tile_pool(name='xin', bufs=2) as xin, \
                 tc.tile_pool(name='xbp', bufs=2) as xbp, \
                 tc.tile_pool(name='xtp', bufs=2) as xtp, \
                 tc.tile_pool(name='qkvp', bufs=3, space='PSUM') as qkvp, \
                 tc.tile_pool(name='raw', bufs=2) as rawp, \
                 tc.tile_pool(name='sqp', bufs=2) as sqp, \
                 tc.tile_pool(name='stp', bufs=2) as stp, \
                 tc.tile_pool(name='nrm', bufs=2) as nrmp, \
                 tc.tile_pool(name='rop', bufs=2) as ropp, \
                 tc.tile_pool(name='ttp', bufs=2) as ttp:
                nc.vector.memset(v_all[:, :, :, HD:HD + 1], 1.0)
                for it in range(TPB):
                    tok0 = b * S + it * 128
                    x_t = xin.tile([128, DIM], F32, tag='x')
                    nc.sync.dma_start(x_t[:], x_ap[tok0:tok0 + 128, :])
                    xb = xbp.tile([128, DIM], BF16, tag='xb')
                    nc.scalar.activation(xb[:], x_t[:], AF.Copy)
                    xT = xtp.tile([128, 8, 128], BF16, tag='xT')
                    nc.sync.dma_start_transpose(xT[:], xb[:])
                    # qkv: 6 psum blocks of 512
                    qraw = rawp.tile([128, DIM], BF16, tag='qraw')
                    kraw = rawp.tile([128, DIM], BF16, tag='kraw')
                    dsts = [(qraw, 0), (qraw, 512), (kraw, 0), (kraw, 512)]
                    for nb in range(6):
                        ps = qkvp.tile([128, 512], F32, tag='ps')
                        # bias enters via a ones-row matmul; the psum drain
                        # becomes a plain cast copy on the idle scalar engine
                        nc.tensor.matmul(ps[:], onesr[:],
                                         bq_sb[:, bass.ts(nb, 512)],
                                         start=True, stop=False)
                        for d in range(8):
                            nc.tensor.matmul(ps[:], xT[:, d, :],
                                             wq_sb[d][:, bass.ts(nb, 512)],
                                             start=False, stop=(d == 7))
                        if nb < 4:
                            dst, off = dsts[nb]
                            nc.scalar.activation(dst[:, off:off + 512], ps[:],
                                                 AF.Copy)
                        else:
                            # v: heads (nb-4)*8 .. +8, strided into v_all
                            h0 = (nb - 4) * 8
                            vd = v_all[:, it, h0:h0 + 8, 0:HD]
                            nc.scalar.activation(
                                vd, ps[:].rearrange('p (h e) -> p h e', h=8),
                                AF.Copy)
                    # rmsnorm: rs = kscale^.5/sqrt(ss + HD*eps)
                    for (raw, ctab, stab, kscale, rtag) in (
                            (qraw, 'cq', 'sq', 1.0, 'q'),
                            (kraw, 'ck', 'sk', 1.0, 'k')):
                        sq = sqp.tile([128, DIM], BF16, tag='sq')
                        nc.gpsimd.tensor_mul(sq[:], raw[:], raw[:])
                        ss = stp.tile([128, H], F32, tag=f'ss{rtag}')
                        nc.vector.tensor_reduce(
                            ss[:], sq[:].rearrange('p (h e) -> p h e', h=H),
                            mybir.AxisListType.X, mybir.AluOpType.add)
                        nc.vector.tensor_scalar_add(ss[:], ss[:], HD * EPS)
                        nc.vector.reciprocal(ss[:], ss[:])
                        rs = stp.tile([128, H], F32, tag=f'rs{rtag}')
                        nc.scalar.activation(rs[:], ss[:], AF.Sqrt, scale=kscale)
                        # normed = raw * rs (free-dim broadcast) -> bf16
                        nn = nrmp.tile([128, DIM], BF16, tag=f'nn{rtag}')
                        rsv = rs[:].unsqueeze(2).broadcast_to([128, H, HD])
                        nc.vector.tensor_mul(
                            nn[:].rearrange('p (h e) -> p h e', h=H),
                            raw[:].rearrange('p (h e) -> p h e', h=H), rsv)
                        # rope (norm weights folded into tables); views are
                        # [128, 2, 8, *] so the 8-head table broadcasts via
                        # a stride-0 dim
                        n4 = nn[:].rearrange('p (g h e) -> p g h e', g=2, h=8)
                        ro = ropp.tile([128, DIM], BF16, tag=f'ro{rtag}')
                        r4 = ro[:].rearrange('p (g h e) -> p g h e', g=2, h=8)
                        t2 = ropp.tile([128, DIM], BF16, tag='t2')
                        t4 = t2[:].rearrange('p (g h e) -> p g h e', g=2, h=8)
                        cosv = rope_sb[ctab][:, it, :] \
                            .rearrange('p (h e) -> p h e', h=8) \
                            .unsqueeze(1).broadcast_to([128, 2, 8, HD])
                        sinv = rope_sb[stab][:, it, :] \
                            .rearrange('p (h e) -> p h e', h=8) \
                            .unsqueeze(1).broadcast_to([128, 2, 8, HD])
                        nc.vector.tensor_mul(r4, n4, cosv)
                        nc.vector.tensor_mul(t4[:, :, :, 0:RD],
                                             n4[:, :, :, RD:HD],
                                             sinv[:, :, :, 0:RD])
                        nc.vector.tensor_mul(t4[:, :, :, RD:HD],
                                             n4[:, :, :, 0:RD],
                                             sinv[:, :, :, RD:HD])
                        nc.gpsimd.tensor_add(ro[:], ro[:], t2[:])
                        # transpose to [feat, tok]; relayout on idle gpsimd
                        tt = ttp.tile([128, 8, 128], BF16, tag='tt')
                        nc.sync.dma_start_transpose(tt[:], ro[:])
                        dstT = qT_all if rtag == 'q' else kT_all
                        nc.gpsimd.tensor_copy(dstT[:, :, it, :], tt[:])

            # ---- P2: attention for this batch (flat SW pipeline: QK/exp of
            # group n is emitted before PV of group n-1 so the PE never
            # waits on the ACT exp of the group it is about to consume) ----
            with tc.tile_pool(name='sps', bufs=2, space='PSUM') as spsp, \
                 tc.tile_pool(name='pvp', bufs=2, space='PSUM') as pvp, \
                 tc.tile_pool(name='ptp', bufs=3) as ptp, \
                 tc.tile_pool(name='rzp', bufs=2) as rzp, \
                 tc.tile_pool(name='aop', bufs=2) as aop:

                def flush(pend):
                    pv, pt, jg, h, ic = pend
                    for i, j in enumerate(jg):
                        nc.tensor.matmul(pv[0:HD + 1, :], v_all[:, j, h, :],
                                         pt[:, bass.ts(i, 512)],
                                         start=(j == 0), stop=(j == 7))
                    if jg[-1] == 7:
                        pb, off = h >> 1, (h & 1) * 64
                        den_b = rzp.tile([1, 512], BF16, tag='den')
                        nc.vector.tensor_copy(den_b[:], pv[HD:HD + 1, :])
                        # broadcast den over 64 rows into pv's spare bank half
                        nc.tensor.matmul(pv[64:128, :], ones_b[:], den_b[:],
                                         start=True, stop=True)
                        # reciprocal_approx_fast requires an offset-0 input;
                        # stage the den broadcast into SBUF first
                        rcs = rzp.tile([HD, 512], F32, tag='rcs')
                        nc.vector.tensor_copy(rcs[:], pv[64:128, :])
                        rcp = rzp.tile([HD, 512], F32, tag='rcp')
                        nc.vector.reciprocal_approx_fast(rcp[:], rcs[:])
                        ao = aop.tile([HD, 512], BF16, tag='ao')
                        nc.vector.tensor_mul(ao[:], pv[0:HD, :], rcp[:])
                        dst = attnT_d[b, pb, ic * 4:(ic + 1) * 4,
                                      off:off + 64, :].transpose([1, 0, 2])
                        nc.gpsimd.dma_start(
                            dst, ao[:].rearrange('p (i t) -> p i t', i=4))

                pend = None
                for h in range(H):
                    pb, off = h >> 1, (h & 1) * 64
                    kT_h = kT_all[off:off + 64, pb, :, :]   # [64, 8(it), 128]
                    qT_h = qT_all[off:off + 64, pb, :, :]
                    for ic in range(2):
                        rhs = qT_h[:, ic * 4:(ic + 1) * 4, :]  # contiguous 512
                        pv = pvp.tile([128, 512], F32, tag='pv')
                        for jg in ((0, 1, 2), (3, 4, 5), (6, 7)):
                            w = len(jg) * 512
                            sps = spsp.tile([128, 1536], F32, tag='sps')
                            for i, j in enumerate(jg):
                                nc.tensor.matmul(
                                    sps[:, bass.ts(i, 512)], kT_h[:, j, :],
                                    rhs, start=True, stop=True)
                            pt = ptp.tile([128, 1536], BF16, tag='pt')
                            nc.scalar.activation(pt[:, 0:w], sps[:, 0:w], AF.Exp)
                            if pend is not None:
                                flush(pend)
                            pend = (pv, pt, jg, h, ic)
                flush(pend)

            run_p3(b)
    nc.compile()
    return nc


_NC_CACHE = None
TRACE = False
LAST_RESULT = None


def _host_tables(w_qkv, b_qkv, q_norm_w, k_norm_w, w_out, b_out):
    """Precompute bf16 weights, biases and folded rope tables."""
    bf = ml_dtypes.bfloat16
    cos, sin = _rope_tables()                     # [S, HD] f32
    rot = np.concatenate([np.arange(RD, HD), np.arange(0, RD)])  # rot-half idx
    sgn = np.concatenate([-np.ones(RD, np.float32), np.ones(RD, np.float32)])

    def fold(nw, scale):
        w = nw.astype(np.float32) * scale
        cosW = cos * w[None, :]                   # [S, HD]
        sinW = sin * sgn[None, :] * w[rot][None, :]
        out = []
        for t in (cosW, sinW):
            t = t.reshape(TPB, 128, HD).transpose(1, 0, 2)    # [128, TPB, HD]
            t = np.broadcast_to(t[:, :, None, :], (128, TPB, 8, HD))
            out.append(np.ascontiguousarray(
                t.reshape(128, TPB, 8 * HD)).astype(bf))
        return out

    cosq, sinq = fold(q_norm_w, 1.0)
    cosk, sink = fold(k_norm_w, 8.0)
    return {
        'wq_b': np.ascontiguousarray(w_qkv, dtype=np.float32).astype(bf),
        'wo_b': np.ascontiguousarray(w_out, dtype=np.float32).astype(bf),
        'bq_b': b_qkv[None, :].astype(np.float32).astype(bf),
        'bo_b': b_out[None, :].astype(np.float32).astype(bf),
        'cosq_b': cosq, 'sinq_b': sinq, 'cosk_b': cosk, 'sink_b': sink,
    }


def kernel(x, w_qkv, b_qkv, q_norm_w, k_norm_w, w_out, b_out):
    global _NC_CACHE, LAST_RESULT
    if _NC_CACHE is None:
        _NC_CACHE = build_graph()
    nc = _NC_CACHE
    com = _host_tables(w_qkv, b_qkv, q_norm_w, k_norm_w, w_out, b_out)
    x = np.ascontiguousarray(x, dtype=np.float32)
    in_maps = []
    for c in range(NCORES):
        m = dict(com)
        m['x'] = np.ascontiguousarray(x[c * BL:(c + 1) * BL].reshape(T, DIM))
        in_maps.append(m)
    res = run_bass_kernel_spmd(nc, in_maps, core_ids=list(range(NCORES)),
                               trace=TRACE)
    LAST_RESULT = res
    outs = [res.results[c]['out'].reshape(BL, S, DIM) for c in range(NCORES)]
    return np.concatenate(outs, axis=0)
